# revision 64
# baseline (speedup 1.0000x reference)
"""Trainium2 Bass kernel for nn_BlockSelfAttention (attention over 8 heads per token).

Math per token t (32768 tokens total, 1024 features = 8 heads x 128 dims):
  xh = x[t].reshape(8, 128)                     # (h, d)
  q = xh @ Wq.T + bq ; k = xh @ Wk.T + bk ; v = xh @ Wv.T + bv
  scores = (q @ k.T) / sqrt(128)                # (8, 8) attention over heads
  out[t] = softmax(scores, -1) @ v              # -> reshape back to 1024

Identities used:
  * bk drops out (adds a per-row constant to scores -> softmax invariant).
  * 1/sqrt(d) and bq are folded into Wq/bq on the host (z-projection:
    scoresT[(t,g),(t,h)] = x_g . z_h with z = s*(Wq^T Wk) x + s*Wk^T bq).
  * bv is added to v rows; since softmax rows sum to 1 the output gets +bv.

Layout strategy (per core: 4096 tokens = 32 tiles of 128 tokens):
  * host pre-transposes x into xt[d, t*8+h]; each SBUF tile XT [d=128, 1024]
    holds 128 tokens; every 128-column block covers 16 whole tokens.
  * z projection: matmul(lhsT=zmt [d,e], rhs=XT) -> zps; ACT evicts with
    per-partition bias ucol -> zT2 [e, (t*8+h)].
  * v projection: matmul(lhsT=XT block, rhs=wvt) -> vps [(t,g), e]; DVE
    evicts with +bv -> V.
  * scores (transposed): rank-17 mask matmul pre-accumulates -30000 off the
    token-diagonal, then matmul(lhsT=XT block, rhs=zT2 block) -> sps
    [(t,g), j, (t,h)]; ACT exp (no max subtraction; |scores| small) -> PT.
  * denominators: matmul(lhsT=PT_j, rhs=ones) -> dps; DVE reciprocal ->
    rsb [P, 8] (one entry per group).
  * AV: matmul(lhsT=PT_j, rhs=V_j) -> avps [(t,h), e]; DVE scales by rsb
    (free-dim broadcast) and writes bf16; host converts to f32 and
    un-permutes.

Schedule (TimelineSim-guided): software-pipelined across tiles with a
one-iteration skew of the back stage. Loop iteration i issues
  front(i):    XT dma, z-proj, z-evict     (zT2 ready an iter before use)
  back_den(i-2): den matmuls + recip      (ahead of vadd in DVE queue)
  mid(i-1):    v-proj, V-add, mask+scores (1-bank halves), exp
  back(i-2):   AV matmuls, out-scale, dma-out
Steady state: every tile takes exactly 2517 ns = the DVE busy floor
(V-add 1192 + recip 133 + out-scale 1192); ACT runs 2448 (z-evict 1224 +
exp 1224), PE ~2190, DMA ~1480. Engines are the wall: PSUM can only be
read by ACT/DVE (no DMA/GPSIMD access), so the four eviction streams
(z, V, exp, out) bound the kernel.
PSUM (8 banks): shared 1-bank ring rb=4 {z halves, score halves, dps} +
vps 2 + avps 2. Const DMAs are issued from the Pool/ACT sequencers in
first-use order so their DGE configs don't stall the pipeline head.
"""

import numpy as np

HEADS = 8
D = 128
B, N, F = 8, 4096, 1024
NCORES = 8
TOK = (B * N) // NCORES          # tokens per core
P = 128                          # tokens per tile
NT = TOK // P                    # tiles per core
NEG = -30000.0

_NC_CACHE = {}
LABELS = {}
BEST_BUFS = dict(ring2=1, act_j=0, order="dv", head=1, zdve=1, warm_pe=3)


def _lab(inst, label):
    try:
        LABELS[str(inst.ins.name)] = label
    except Exception:
        pass
    return inst


def _build_nc(mm_dt_name="bf16", BUFS=None):
    import concourse.mybir as mybir
    import concourse.tile as tile
    from concourse import bacc
    from contextlib import ExitStack

    f32 = mybir.dt.float32
    bf16 = mybir.dt.bfloat16
    mm_dt = {"f32": f32, "bf16": bf16}[mm_dt_name]

    BUFS = BUFS or {}
    ACT_J = BUFS.get("act_j", 1)       # out-scale slices done on ACT
    RING = BUFS.get("ring", 0)         # shared 1-bank psum ring (v1-style)
    RING2 = BUFS.get("ring2", 0)       # ring for z/s/dps; 2-bank vps/avps
    RB = BUFS.get("rb", 5)             # shared ring depth
    AB = BUFS.get("ab", 3)             # avps ring depth
    if RING2:
        RING, RB = 1, BUFS.get("rb", 4)
    Z_SPLIT = BUFS.get("z_split", 0) or RING
    V_SPLIT = (BUFS.get("v_split", 0) or RING) and not RING2
    SPS_BUFS = BUFS.get("sps_bufs", 2)  # 1-bank score-half buffers
    nc = bacc.Bacc("TRN2", target_bir_lowering=False, debug=False)

    xt = nc.dram_tensor("xt", [D, TOK * HEADS], mm_dt, kind="ExternalInput")
    zmt = nc.dram_tensor("zmt", [D, D], mm_dt, kind="ExternalInput")
    wvt = nc.dram_tensor("wvt", [D, D], mm_dt, kind="ExternalInput")
    ucol = nc.dram_tensor("ucol", [D, 1], f32, kind="ExternalInput")
    bvr = nc.dram_tensor("bvr", [D, D], f32, kind="ExternalInput")
    mka = nc.dram_tensor("mka", [32, D], bf16, kind="ExternalInput")
    mkb = nc.dram_tensor("mkb", [32, 4 * D], bf16, kind="ExternalInput")
    one = nc.dram_tensor("one", [D, 1], mm_dt, kind="ExternalInput")
    y = nc.dram_tensor("y", [TOK, F], bf16, kind="ExternalOutput")

    xt_r = xt.ap().rearrange("d (T c) -> T d c", c=P * HEADS)
    # scrambled tile-row-major dump; host un-permutes
    y_r = y.ap().rearrange("(T p) c -> T p c", p=P)

    AF = mybir.ActivationFunctionType

    with tile.TileContext(nc) as tc, ExitStack() as es:
        cpool = es.enter_context(tc.tile_pool(name="consts", bufs=1))
        zmt_s = cpool.tile([D, D], mm_dt, tag="zmt")
        wvt_s = cpool.tile([D, D], mm_dt, tag="wvt")
        ucol_s = cpool.tile([D, 1], f32, tag="ucol")
        bvr_s = cpool.tile([D, D], f32, tag="bvr")
        mka_s = cpool.tile([32, D], bf16, tag="mka")
        mkb_s = cpool.tile([32, 4 * D], bf16, tag="mkb")
        one_s = cpool.tile([D, 1], mm_dt, tag="one")
        bvr_b = bvr_s[:, None, :].broadcast_to([P, HEADS, D])

        pxt = es.enter_context(tc.tile_pool(name="pxt", bufs=BUFS.get("pxt", 5)))
        pz = es.enter_context(tc.tile_pool(name="pz", bufs=BUFS.get("pz", 4)))
        pv = es.enter_context(tc.tile_pool(name="pv", bufs=BUFS.get("pv", 5)))
        ppt = es.enter_context(tc.tile_pool(name="ppt", bufs=BUFS.get("ppt", 5)))
        pdr = es.enter_context(tc.tile_pool(name="pdr", bufs=BUFS.get("pdr", 4)))
        po = es.enter_context(tc.tile_pool(name="po", bufs=BUFS.get("po", 4)))
        # PSUM: 8 banks total. zps 2 + vps 2 + sps 2 + avps 2; dps carved
        # into the vps tile (WAR-ordered after its eviction).
        pps = es.enter_context(tc.tile_pool(name="pps", bufs=1, space="PSUM"))

        # tile-0 input via the Pool SWDGE path: ~1.1us to first transfer
        # vs ~1.9us through SP's DGE spin-up
        st0 = {}
        if BUFS.get("pool_in0", 0):
            XT0 = pxt.tile([D, P * HEADS], mm_dt, tag="xt", name="XT0")
            _lab(nc.gpsimd.dma_start(XT0[:], xt_r[0]), "dmaIn.0")
            st0[("xt", 0)] = XT0
        # const loads ordered by first use and split across the Pool/ACT
        # sequencers so their DGE configs don't stall the pipeline head
        for t_, d_ in ((zmt_s, zmt), (wvt_s, wvt), (bvr_s, bvr),
                       (one_s, one)):
            nc.gpsimd.dma_start(t_[:], d_.ap())
        for t_, d_ in ((ucol_s, ucol), (mkb_s, mkb), (mka_s, mka)):
            nc.scalar.dma_start(t_[:], d_.ap())

        # warm the ACT exp table while the first DMAs are in flight
        warm = cpool.tile([1, 2], f32, tag="warm")
        nc.gpsimd.memset(warm[:], 0.0)
        nc.scalar.activation(warm[0:1, 0:1], warm[0:1, 1:2], AF.Exp)
        WARM_PE = BUFS.get("warm_pe", 0)
        if WARM_PE:
            # spin the PE during the DMA preamble: the p-state window is
            # 3us from the FIRST PE activity, so starting it early makes
            # all real matmuls run at full clock. DVE memset (Pool's seq
            # is busy with const DGE configs at this point).
            wsrc = cpool.tile([D, 512], bf16, tag="wsrc")
            if BUFS.get("warm_delay", 0):
                # push the p-state window start later so it still covers
                # the first real matmuls (window = first PE activity + 3us)
                wdel = cpool.tile([D, BUFS["warm_delay"]], f32, tag="wdel")
                nc.vector.memset(wdel[:], 0.0)
                nc.vector.memset(wsrc[:], 0.0)
            else:
                nc.vector.memset(wsrc[:], 0.0)
            wps = pps.tile([D, 512], f32, tag="ps", bufs=RB)
            for r in range(WARM_PE):
                _lab(nc.tensor.matmul(wps[:], wsrc[:, 0:D], wsrc[:],
                                      start=True, stop=True,
                                      skip_group_check=True), f"warm{r}")

        # per-stage state passed between pipeline iterations
        st = {}
        st.update(st0)

        def front_in(i, halves=False):
            XT = pxt.tile([D, P * HEADS], mm_dt, tag="xt")
            if halves:
                for half in range(2):
                    csl = slice(512 * half, 512 * half + 512)
                    _lab(nc.sync.dma_start(XT[:, csl], xt_r[i][:, csl]),
                         f"dmaIn{half}.{i}")
            else:
                _lab(nc.sync.dma_start(XT[:], xt_r[i]), f"dmaIn.{i}")
            st[("xt", i)] = XT

        def front(i, skip_in=False):
            # XT dma + z projection + z eviction for tile i
            if not skip_in:
                front_in(i)
            XT = st[("xt", i)]
            zT2 = pz.tile([D, P * HEADS], mm_dt, tag="z")
            if Z_SPLIT:
                for half in range(2):
                    csl = slice(512 * half, 512 * half + 512)
                    zps = pps.tile([D, 512], f32, tag="ps" if RING else "zps",
                                   bufs=RB if RING else None)
                    _lab(nc.tensor.matmul(zps[:], zmt_s[:], XT[:, csl],
                                          start=True, stop=True),
                         f"zmm{half}.{i}")
                    if i < BUFS.get("zdve", 2 if BUFS.get("head", 0) else 0):
                        # head: DVE is idle, ACT is the critical chain
                        _lab(nc.vector.tensor_scalar_add(
                            zT2[:, csl], zps[:], ucol_s[:, 0:1]),
                            f"zevD{half}.{i}")
                    else:
                        _lab(nc.scalar.activation(zT2[:, csl], zps[:],
                                                  AF.Identity,
                                                  bias=ucol_s[:, 0:1]),
                             f"zev{half}.{i}")
            else:
                zps = pps.tile([D, P * HEADS], f32, tag="zps")
                for half in range(2):
                    csl = slice(512 * half, 512 * half + 512)
                    _lab(nc.tensor.matmul(zps[:, csl], zmt_s[:], XT[:, csl],
                                          start=True, stop=True),
                         f"zmm{half}.{i}")
                _lab(nc.scalar.activation(zT2[:], zps[:], AF.Identity,
                                          bias=ucol_s[:, 0:1]), f"zev.{i}")
            st[("z", i)] = zT2

        def mid_s(i):
            # mask + scores + exp for tile i, in two 1-bank halves so the
            # sps tag double-buffers across iterations (bufs=2)
            XT = st[("xt", i)]
            zT2 = st.pop(("z", i))
            PT = ppt.tile([P, HEADS, P], mm_dt, tag="pt")
            for half in range(2):
                sps = pps.tile([P, 4, P], f32, tag="ps" if RING else "sps",
                               bufs=RB if RING else SPS_BUFS)
                _lab(nc.tensor.matmul(sps[:], mka_s[:], mkb_s[:],
                                      start=True, stop=False),
                     f"mask{half}.{i}")
                for jj in range(4):
                    j = 4 * half + jj
                    gsl = slice(128 * j, 128 * j + 128)
                    _lab(nc.tensor.matmul(sps[:, jj, :], XT[:, gsl],
                                          zT2[:, gsl], start=False, stop=True,
                                          skip_group_check=True),
                         f"sc{j}.{i}")
                _lab(nc.scalar.activation(PT[:, 4 * half:4 * half + 4, :],
                                          sps[:], AF.Exp), f"exp{half}.{i}")
            st[("pt", i)] = PT

        def mid_v(i):
            # v projection + bias eviction for tile i (after scores on PE)
            XT = st[("xt", i)]
            V = pv.tile([P, HEADS, D], mm_dt, tag="v")
            if V_SPLIT:
                for half in range(2):
                    hsl = slice(4 * half, 4 * half + 4)
                    vps = pps.tile([P, 4, D], f32,
                                   tag="ps" if (RING and not RING2) else "vps",
                                   bufs=RB if (RING and not RING2) else None)
                    for jj in range(4):
                        j = 4 * half + jj
                        _lab(nc.tensor.matmul(vps[:, jj, :],
                                              XT[:, 128 * j:128 * j + 128],
                                              wvt_s[:], start=True,
                                              stop=True), f"v{j}.{i}")
                    _lab(nc.vector.tensor_add(V[:, hsl, :], vps[:],
                                              bvr_s[:, None, :].broadcast_to(
                                                  [P, 4, D])),
                         f"vadd{half}.{i}")
            else:
                vps = pps.tile([P, HEADS, D], f32, tag="vps")
                for j in range(HEADS):
                    _lab(nc.tensor.matmul(vps[:, j, :],
                                          XT[:, 128 * j:128 * j + 128],
                                          wvt_s[:], start=True, stop=True),
                         f"v{j}.{i}")
                _lab(nc.vector.tensor_add(V[:], vps[:], bvr_b), f"vadd.{i}")
            st[("v", i)] = V

        def back_den(i):
            # denominators for tile i; ring mode: own 1-bank alloc,
            # otherwise carved into the avps j=7 corner (av j=7 issued
            # last, resets it after recip has read it)
            PT = st[("pt", i)]
            if RING2 and BUFS.get("rpair", 0):
                # pair tiles (2k, 2k+1) in one dps slot and one reciprocal:
                # the 125ns DVE init amortizes over 16 lanes instead of 8
                if i % 2 == 0:
                    dt_ = pps.tile([P, 2 * HEADS], f32, tag="ps", bufs=RB,
                                   name="dps2")
                    st[("dps2", i)] = dt_
                    dps = dt_[:, 0:HEADS]
                else:
                    dt_ = st.pop(("dps2", i - 1))
                    dps = dt_[:, HEADS:2 * HEADS]
                avps = pps.tile([P, HEADS, D], f32, tag="avps")
                for j in range(HEADS):
                    _lab(nc.tensor.matmul(dps[:, j:j + 1], PT[:, j, :],
                                          one_s[:], start=True, stop=True,
                                          skip_group_check=True),
                         f"den{j}.{i}")
                if i % 2 == 0:
                    st[("rsb_pend", i)] = dt_
                    rsb = None
                else:
                    rsb2 = pdr.tile([P, 2 * HEADS], f32, tag="rs",
                                    name="rsb2")
                    _lab(nc.vector.reciprocal(rsb2[:], dt_[:]),
                         f"recip2.{i}")
                    st[("rsb", i - 1)] = rsb2[:, 0:HEADS]
                    st[("rsb", i)] = rsb2[:, HEADS:2 * HEADS]
                st[("avps", i)] = avps
                return
            if RING2:
                dt_ = pps.tile([P, HEADS], f32, tag="ps", bufs=RB)
                dps = dt_[:]
                avps = pps.tile([P, HEADS, D], f32, tag="avps")
            elif RING:
                dt_ = pps.tile([P, HEADS], f32, tag="ps", bufs=RB)
                dps = dt_[:]
                avps = None
            else:
                avps = pps.tile([P, HEADS, D], f32, tag="avps")
                dps = avps[:, HEADS - 1, 0:HEADS]
            for j in range(HEADS):
                _lab(nc.tensor.matmul(dps[:, j:j + 1], PT[:, j, :], one_s[:],
                                      start=True, stop=True,
                                      skip_group_check=True), f"den{j}.{i}")
            rsb = pdr.tile([P, HEADS], f32, tag="rs")
            _lab(nc.vector.reciprocal(rsb[:], dps), f"recip.{i}")
            st[("avps", i)] = avps
            st[("rsb", i)] = rsb

        def back_av(i):
            PT = st.pop(("pt", i))
            V = st.pop(("v", i))
            if RING and not RING2:
                halves = []
                for half in range(2):
                    avp = pps.tile([P, 4, D], f32, tag="av", bufs=AB)
                    for jj in range(4):
                        j = 4 * half + jj
                        _lab(nc.tensor.matmul(avp[:, jj, :], PT[:, j, :],
                                              V[:, j, :], start=True,
                                              stop=True), f"av{j}.{i}")
                    halves.append(avp)
                st[("avps", i)] = halves
                return
            avps = st[("avps", i)]
            if RING2:
                for j in range(HEADS):
                    _lab(nc.tensor.matmul(avps[:, j, :], PT[:, j, :],
                                          V[:, j, :], start=True, stop=True),
                         f"av{j}.{i}")
                return
            for j in range(HEADS - 1):
                _lab(nc.tensor.matmul(avps[:, j, :], PT[:, j, :], V[:, j, :],
                                      start=True, stop=True), f"av{j}.{i}")
            # j=7 last: WAR on the dps corner after recip has read it
            _lab(nc.tensor.matmul(avps[:, 7, :], PT[:, 7, :], V[:, 7, :],
                                  start=True, stop=True,
                                  skip_group_check=True), f"av7.{i}")

        def back_out(i):
            avps = st.pop(("avps", i))
            rsb = st.pop(("rsb", i))
            out = po.tile([P, HEADS, D], bf16, tag="o")
            if RING and not RING2:
                ha, hb = avps
                nj = 4 - ACT_J   # DVE js in half b
                _lab(nc.vector.tensor_mul(
                    out[:, 0:4, :], ha[:],
                    rsb[:, 0:4, None].broadcast_to([P, 4, D])), f"omulA.{i}")
                if nj:
                    _lab(nc.vector.tensor_mul(
                        out[:, 4:4 + nj, :], hb[:, 0:nj, :],
                        rsb[:, 4:4 + nj, None].broadcast_to([P, nj, D])),
                        f"omulB.{i}")
                for jj in range(nj, 4):
                    _lab(nc.scalar.activation(out[:, 4 + jj, :],
                                              hb[:, jj, :], AF.Copy,
                                              scale=rsb[:, 4 + jj:5 + jj]),
                         f"oact{4+jj}.{i}")
                if i == NT - 1:
                    orr = out[:].rearrange("p j e -> p (j e)")
                    for half in range(2):
                        csl = slice(512 * half, 512 * half + 512)
                        _lab(nc.sync.dma_start(y_r[i][:, csl], orr[:, csl]),
                             f"dmaOut{half}.{i}")
                else:
                    _lab(nc.sync.dma_start(
                        y_r[i], out[:].rearrange("p j e -> p (j e)")),
                        f"dmaOut.{i}")
                return
            k = HEADS - ACT_J
            if i == NT - 1 and k == HEADS:
                # last tile: split scale+dma into halves so the final DMA
                # chain starts a half-omul earlier
                orr = out[:].rearrange("p j e -> p (j e)")
                for half in range(2):
                    hsl = slice(4 * half, 4 * half + 4)
                    csl = slice(512 * half, 512 * half + 512)
                    _lab(nc.vector.tensor_mul(
                        out[:, hsl, :], avps[:, hsl, :],
                        rsb[:, hsl, None].broadcast_to([P, 4, D])),
                        f"omul{half}.{i}")
                    _lab(nc.sync.dma_start(y_r[i][:, csl], orr[:, csl]),
                         f"dmaOut{half}.{i}")
                return
            if k:
                _lab(nc.vector.tensor_mul(
                    out[:, 0:k, :], avps[:, 0:k, :],
                    rsb[:, 0:k, None].broadcast_to([P, k, D])), f"omul.{i}")
            for j in range(k, HEADS):
                _lab(nc.scalar.activation(out[:, j, :], avps[:, j, :],
                                          AF.Copy, scale=rsb[:, j:j + 1]),
                     f"oact{j}.{i}")
            _lab(nc.sync.dma_start(y_r[i],
                                   out[:].rearrange("p j e -> p (j e)")),
                 f"dmaOut.{i}")

        ORDER = BUFS.get("order", "vs")
        HEAD = BUFS.get("head", 0) and ORDER == "dv"
        start_i = 0
        if HEAD:
            # depth-first priorities for the pipeline head: tile 0's
            # chain outranks tile 1/2 work so the first omul lands early
            if not BUFS.get("pool_in0", 0):
                front_in(0)
            front_in(1)
            front(0, skip_in=True)
            front_in(2)
            mid_v(0)
            front(1, skip_in=True)
            mid_s(0)
            front_in(3)
            back_den(0)
            front(2, skip_in=True)
            mid_s(1)
            back_av(0)
            if not BUFS.get("rpair", 0):
                back_out(0)
            mid_v(1)
            start_i = 3
        for i in range(start_i, NT + 3):
            if i < NT:
                front(i, skip_in=i < 4 and HEAD)
            if ORDER == "dv":
                if 2 <= i < NT + 2 and not (HEAD and i - 2 < 1):
                    back_den(i - 2)
                if 1 <= i < NT + 1 and not (HEAD and i - 1 < 2):
                    mid_v(i - 1)
                    mid_s(i - 1)
                if 2 <= i < NT + 2 and not (HEAD and i - 2 < 1):
                    back_av(i - 2)
                    if BUFS.get("rpair", 0):
                        if (i - 2) % 2 == 1:
                            back_out(i - 3)
                            back_out(i - 2)
                    else:
                        back_out(i - 2)
            elif ORDER == "vs":
                if i >= 3:
                    back_out(i - 3)
                if 1 <= i < NT + 1:
                    mid_v(i - 1)
                    mid_s(i - 1)
                if 2 <= i < NT + 2:
                    back_den(i - 2)
                    back_av(i - 2)
            else:
                if 1 <= i < NT + 1:
                    mid_s(i - 1)
                if 2 <= i < NT + 2:
                    back_den(i - 2)
                if 1 <= i < NT + 1:
                    mid_v(i - 1)
                if 2 <= i < NT + 2:
                    back_av(i - 2)
                if i >= 3:
                    back_out(i - 3)

    nc.compile()
    return nc


def _get_nc(mm_dt_name="bf16", BUFS=None):
    key = (mm_dt_name, tuple(sorted((BUFS or {}).items())))
    if key not in _NC_CACHE:
        _NC_CACHE[key] = _build_nc(mm_dt_name, BUFS)
    return _NC_CACHE[key]


def _prep_in_maps(x, Wq, bq, Wk, bk, Wv, bv, mm_dt_name="bf16"):
    import ml_dtypes
    mm_np = ml_dtypes.bfloat16 if mm_dt_name == "bf16" else np.float32
    s = np.float32(1.0 / np.sqrt(D))
    Wq = np.asarray(Wq, np.float64)
    Wk = np.asarray(Wk, np.float64)
    zmt = np.ascontiguousarray(s * (Wq.T @ Wk)).astype(mm_np)
    ucol = (s * (Wk.T @ np.asarray(bq, np.float64))).reshape(D, 1).astype(
        np.float32)
    wvt = np.ascontiguousarray(np.asarray(Wv).T).astype(mm_np)
    bvr = np.tile(np.asarray(bv).reshape(1, D).astype(np.float32), (D, 1))
    a = np.float32(np.sqrt(-NEG))
    mka = np.zeros((32, D), np.float32)
    mkb = np.zeros((32, D), np.float32)
    mka[0, :] = a
    mkb[0, :] = -a
    for j in range(16):
        mka[1 + j, 8 * j:8 * j + 8] = a
        mkb[1 + j, 8 * j:8 * j + 8] = a
    mka = mka.astype(ml_dtypes.bfloat16)
    mkb = np.tile(mkb, (1, 4)).astype(ml_dtypes.bfloat16)
    one = np.ones((D, 1), mm_np)
    xs = np.asarray(x, np.float32).reshape(B * N, F)
    shared = dict(zmt=zmt, wvt=wvt, ucol=ucol, bvr=bvr, mka=mka,
                  mkb=mkb, one=one)
    in_maps = []
    for c in range(NCORES):
        xc = xs[c * TOK:(c + 1) * TOK]
        # xt[d, t*8+h] = x[t, h*128+d]
        xtc = np.ascontiguousarray(
            xc.reshape(TOK, HEADS, D).transpose(2, 0, 1).reshape(
                D, TOK * HEADS)).astype(mm_np)
        in_maps.append(dict(xt=xtc, **shared))
    return in_maps


def run(x, Wq, bq, Wk, bk, Wv, bv, mm_dt_name="bf16", run_bufs=None,
        **run_kw):
    from concourse.bass_utils import run_bass_kernel_spmd

    if run_bufs is None:
        run_bufs = BEST_BUFS
    nc = _get_nc(mm_dt_name, run_bufs)
    in_maps = _prep_in_maps(x, Wq, bq, Wk, bk, Wv, bv, mm_dt_name)
    res = run_bass_kernel_spmd(nc, in_maps, core_ids=list(range(NCORES)),
                               **run_kw)
    yl = []
    for c in range(NCORES):
        a = np.asarray(res.results[c]["y"], np.float32)
        # un-scramble: rows of each 128-token tile are (t%16)*8+h
        a = a.reshape(NT, 16, 8, 8, D).transpose(0, 3, 1, 2, 4).reshape(
            TOK, F)
        yl.append(a)
    yv = np.concatenate(yl, axis=0).reshape(B, N, F)
    return yv, res


def kernel(x, Wq, bq, Wk, bk, Wv, bv):
    yv, _ = run(x, Wq, bq, Wk, bk, Wv, bv, mm_dt_name="bf16",
                run_bufs=BEST_BUFS)
    return yv
